# revision 32
# baseline (speedup 1.0000x reference)
"""Trainium2 Bass kernel for nn_AEQMPatchSegmModel (retrieval_knn).

Strategy
--------
Data-parallel over batch: 16 images / 8 cores = 2 images per core.

The per-patch encoder (bilinear resize 8->16 + three stride-2 SAME convs +
dense) is an alternation of LINEAR maps and relus.  Each linear stage is
folded (on host, exactly) into a position-blocked matrix:

    A1 [2048,192]  = conv1 o resize   (patch pixels -> 8x8x32 pre-relu)
    A2 [1024,2048] = conv2            (8x8x32 -> 4x4x64 pre-relu)
    A3 [ 512,1024] = conv3            (4x4x64 -> 2x2x128 pre-relu)
    wd [ 512,  64] = dense

Conv locality makes these matrices block-sparse: an output row-block only
depends on 2-3 input row-blocks.  The device kernel runs a static schedule
of 128-wide accumulating matmuls over just the nonzero blocks, with
patches (n-tiles of 405 = 9 patch-rows x 45) on the matmul free dim.

The RBF head is folded into one augmented matmul: V_aug = [v; v^2] (the
dense matmul emits v twice; one half is squared with bias bd), then
  t[k,n] = sum_d 2*c_x[k,d]/s2 * v - 1/s2 * v^2  (+ bias: -|c_x|^2/s2 + log w_k)
  ow = exp(t) = comp_w * K^2;   sum_n max(ow,EPS) = sum_n relu(ow-EPS) + n*EPS
Per-core output: S [128 comps, 2 images].  Host: + n*EPS, normalize,
project through normalize(c_y)^2 -> (16,10).

Patch extraction: images are DMA'd once into a column-phase-expanded SBUF
layout IMG[(j,ci), rr, parity, px] = img[2*rr+parity, 2*px+j, ci]; every
A1 matmul rhs is then a pure strided view (no im2col materialization).
"""

import numpy as np

EPS = 1e-10
NTILE = 405          # 9 patch-rows x 45 patch-cols per n-tile
NT = 5               # n-tiles per image
NPI = 2025           # patches per image
F32 = np.float32

_CACHE = {}


# ---------------------------------------------------------------- host math
def _resize_mat():
    R = np.zeros((16, 8), np.float64)
    for k in range(16):
        x = (k + 0.5) / 2.0 - 0.5
        x0 = int(np.floor(x))
        t = x - x0
        i0 = min(max(x0, 0), 7)
        i1 = min(max(x0 + 1, 0), 7)
        R[k, i0] += 1.0 - t
        R[k, i1] += t
    return R


def _conv_s2_same_jac(H, w):
    """Jacobian of jax stride-2 SAME 3x3 conv on (H,H,Cin) -> (H/2,H/2,Cout).

    XLA SAME for even H, stride 2, k=3: pad_total=1 -> pad_lo=0, pad_hi=1,
    so input row of output oy, tap ky is iy = 2*oy + ky (dropped if iy >= H).
    """
    kh, kw, Cin, Cout = w.shape
    Ho = H // 2
    J = np.zeros((Ho, Ho, Cout, H, H, Cin), np.float64)
    for ky in range(3):
        for kx in range(3):
            for oy in range(Ho):
                iy = 2 * oy + ky
                if iy >= H:
                    continue
                for ox in range(Ho):
                    ix = 2 * ox + kx
                    if ix >= H:
                        continue
                    J[oy, ox, :, iy, ix, :] += w[ky, kx].T
    return J.reshape(Ho * Ho * Cout, H * H * Cin)


def _a2_schedule():
    """[(mtile_index, kt1, my, p, jy, hh)] for conv2 block matmuls."""
    sched = []
    for my in range(4):
        for p in range(2):
            mt2 = my * 2 + p
            jys = [jy for jy in (2 * my, 2 * my + 1, 2 * my + 2) if jy <= 7]
            halves = (0, 1) if p == 0 else (1,)
            for jy in jys:
                for hh in halves:
                    sched.append((mt2, jy * 2 + hh, my, p, jy, hh))
    return sched


def _a3_schedule():
    """[(mtile_index, kt2, ny, nx, my, p)] for conv3 block matmuls."""
    sched = []
    for ny in range(2):
        for nx in range(2):
            mt3 = ny * 2 + nx
            mys = [my for my in (2 * ny, 2 * ny + 1, 2 * ny + 2) if my <= 3]
            pairs = (0, 1) if nx == 0 else (1,)
            for my in mys:
                for p in pairs:
                    sched.append((mt3, my * 2 + p, ny, nx, my, p))
    return sched


def _build_consts(w1, b1, w2, b2, w3, b3, wd, bd, c_x, c_y, comp_w, sigma):
    R = _resize_mat()
    RZ = np.kron(np.kron(R, R), np.eye(3))                      # (768,192)
    A1 = _conv_s2_same_jac(16, np.asarray(w1, np.float64)) @ RZ  # (2048,192)
    A2 = _conv_s2_same_jac(8, np.asarray(w2, np.float64))        # (1024,2048)
    A3 = _conv_s2_same_jac(4, np.asarray(w3, np.float64))        # (512,1024)

    A1r = A1.reshape(8, 8, 32, 8, 8, 3)      # (jy,jx,co, i,j,ci)
    a1c = np.zeros((25, 8, 2, 3, 128), np.float64)   # row 24: conv1 bias (slot 0)
    chk = np.zeros_like(A1r)
    for jy in range(8):
        il = [i for i in (jy - 1, jy, jy + 1) if 0 <= i <= 7]
        for h in range(2):
            for s, i in enumerate(il):
                blk = A1r[jy, 4 * h:4 * h + 4, :, i, :, :]       # (jx,co,j,ci)
                a1c[:24, jy, h, s, :] = blk.transpose(2, 3, 0, 1).reshape(24, 128)
                chk[jy, 4 * h:4 * h + 4, :, i, :, :] = blk
            a1c[24, jy, h, 0, :] = np.tile(np.asarray(b1, np.float64), 4)
    assert np.abs(A1r - chk).max() < 1e-12, "A1 support mismatch"

    s2 = _a2_schedule()
    A2r = A2.reshape(4, 4, 64, 8, 8, 32)     # (my,mx,co, jy,jx,ci)
    a2c = np.zeros((128, len(s2), 128), np.float64)
    chk = np.zeros_like(A2r)
    for n, (mt2, kt1, my, p, jy, hh) in enumerate(s2):
        blk = A2r[my, 2 * p:2 * p + 2, :, jy, 4 * hh:4 * hh + 4, :]  # (mx,co,jx,ci)
        a2c[:, n, :] = blk.transpose(2, 3, 0, 1).reshape(128, 128)
        chk[my, 2 * p:2 * p + 2, :, jy, 4 * hh:4 * hh + 4, :] = blk
    assert np.abs(A2r - chk).max() < 1e-12, "A2 support mismatch"

    s3 = _a3_schedule()
    A3r = A3.reshape(2, 2, 128, 4, 4, 64)    # (ny,nx,co, my,mx,ci)
    a3c = np.zeros((128, len(s3), 128), np.float64)
    chk = np.zeros_like(A3r)
    for n, (mt3, kt2, ny, nx, my, p) in enumerate(s3):
        blk = A3r[ny, nx, :, my, 2 * p:2 * p + 2, :]             # (co,mx,ci)
        a3c[:, n, :] = blk.transpose(1, 2, 0).reshape(128, 128)
        chk[ny, nx, :, my, 2 * p:2 * p + 2, :] = blk
    assert np.abs(A3r - chk).max() < 1e-12, "A3 support mismatch"

    wdm = np.asarray(wd, np.float64).reshape(4, 128, 64)         # (pos,c3,enc)
    wd2 = np.zeros((128, 4, 128), np.float64)
    wd2[:, :, 0:64] = wdm.transpose(1, 0, 2)
    wd2[:, :, 64:128] = wdm.transpose(1, 0, 2)

    # Fold the dense bias into the RBF head: with u = wd.T@h3 (pre-bias),
    # v = u + bd, so d2 = |u - (c_x - bd)|^2.  The device then never adds
    # bd; vaug = [u; u^2] and e := c_x - bd replaces c_x below.
    sig2 = float(np.asarray(sigma, np.float64) ** 2)
    e = np.asarray(c_x, np.float64) - np.asarray(bd, np.float64)[None, :]
    rbfw = np.zeros((128, 128), np.float64)
    rbfw[0:64, :] = (2.0 / sig2) * e.T
    rbfw[64:128, :] = -1.0 / sig2
    biasr = (-(e ** 2).sum(-1) / sig2
             + np.log(np.asarray(comp_w, np.float64)))[:, None]  # (128,1)

    bcol = np.zeros((1, 3, 128), np.float64)
    bcol[0, 0, :] = np.tile(np.asarray(b2, np.float64), 2)
    bcol[0, 1, :] = np.asarray(b3, np.float64)
    bcol[0, 2, :] = biasr[:, 0]
    c = {
        "a1c": a1c, "a2c": a2c, "a3c": a3c, "wd2": wd2, "rbfw": rbfw,
        "bcol": bcol,
    }
    return {k: np.ascontiguousarray(v, F32) for k, v in c.items()}, s2, s3


# ---------------------------------------------------------------- device
def _build_nc(n2, n3):
    import concourse.bass as bass
    import concourse.mybir as mybir
    import concourse.tile as tile
    import concourse.tile_sem_assignment as tsa
    tsa.NUM_HWDGE_SEMS = 1   # all HWDGE DMAs share one sem (kernel-tail
    #                          Drain has a tiny sync-wait budget in codegen)
    from concourse.vector_clock import ScopedClock

    def _split_drain_and_barrier(self, tick_clock, wait_clock):
        # codegen allows ~1 sync-wait per instruction; the stock tail drain
        # carries one wait per live semaphore.  Emit standalone SP waits
        # instead (drain first, then waits, then barrier — same net sync).
        bnc = self.nc
        drain_inst = bnc.sync.drain()
        wait_clock.add_sem_waits(
            drain_inst.ins, ScopedClock({None: tick_clock.global_clock})
        )
        si = drain_inst.ins.sync_info
        waits = list(si.on_wait) if si is not None and si.on_wait else []
        if len(waits) > 1:
            try:
                si.on_wait = waits[:0]
            except Exception:
                drain_inst.ins.sync_info = None
            num2sem = {s.num: s for s in self.sems.allocated().values()}
            for w in waits:
                bnc.sync.wait_ge(num2sem[int(w.id)], int(w.wait_value))
        bnc.all_engine_barrier()
        assert self.sems is not None
        popped = bnc._tile_sem_poison_stack.pop()
        assert popped is self._sem_poison
        bnc.clear_and_free_semaphores(list(self.sems.allocated().values()))
        bnc.all_engine_barrier()

    tile.TileContext._drain_and_barrier = _split_drain_and_barrier

    f32 = mybir.dt.float32
    AF = mybir.ActivationFunctionType
    nc = bass.Bass()
    _negeps = nc.alloc_sbuf_tensor("const-float32-negeps", [128, 1], f32)
    nc.gpsimd.memset(_negeps.ap(), -EPS)
    nc.const_aps.aps[(f32, -EPS)] = _negeps.ap()
    nc.all_engine_barrier()
    S_raw = nc.alloc_sbuf_tensor("Sout", [128, 2], f32)

    W1 = (n2 + n3 + 4 + 1) * 128 + 384 + 405   # wblob cols
    W2 = 8 * 2 * 3 * 128 + 2 * 48 * 2 * 45     # iblob cols
    wblob_d = nc.declare_dram_parameter("wblob", [128, W1], f32, isOutput=False)
    iblob_d = nc.declare_dram_parameter("iblob", [25, W2], f32, isOutput=False)
    out_d = nc.declare_dram_parameter("out", [128, 2], f32, isOutput=True)

    s2 = _a2_schedule()
    s3 = _a3_schedule()

    with tile.TileContext(nc) as tc:
        with (
            tc.tile_pool(name="w", bufs=1) as wpool,
            tc.tile_pool(name="act", bufs=1) as apool,
            tc.tile_pool(name="sm", bufs=3) as spool,
            tc.tile_pool(name="ps", bufs=7, space="PSUM") as ppool,
        ):
            wblob = wpool.tile([128, W1], f32)
            nc.sync.dma_start(wblob[:], wblob_d[:])
            iblob = wpool.tile([25, W2], f32, tag="iblob")
            nc.sync.dma_start(iblob[:], iblob_d[:])
            o = 0
            a2 = wblob[:, o:o + n2 * 128].rearrange("p (n k) -> p n k", n=n2)
            o += n2 * 128
            a3 = wblob[:, o:o + n3 * 128].rearrange("p (n k) -> p n k", n=n3)
            o += n3 * 128
            wdt = wblob[:, o:o + 512].rearrange("p (n k) -> p n k", n=4)
            o += 512
            rbf = wblob[:, o:o + 128]
            o += 128
            bcol = wblob[0:1, o:o + 384].rearrange("p (a b) -> p a b", a=3)
            o += 384
            onest = wblob[0:1, o:o + 405]
            # iblob: a1c [25,8,2,3,128] then IMG [25,2,48,2,45]
            # IMG[(j,ci), g, rr, par, px] = image[g, 2rr+par, 2px+j, ci]
            a1 = iblob[:, 0:6144].rearrange(
                "p (a b c d) -> p a b c d", a=8, b=2, c=3)
            img = iblob[:, 6144:].rearrange(
                "p (g rr par px) -> p g rr par px", g=2, rr=48, par=2)

            red = apool.tile([128, 2, 5], f32, tag="red")

            # PE pre-touch: dummy 1x1x1 matmuls so the PE vector clock
            # observes every load-DMA queue before the real matmuls
            # (PE LDWEIGHTS supports only ONE sync-wait slot in codegen).
            dps = ppool.tile([1, 1], f32, tag="dps", bufs=1)
            for dummy in (wblob[0:1, 0:1], iblob[0:1, 0:1]):
                nc.tensor.matmul(dps[:], dummy, dummy, start=True, stop=True)
            ones_rhs = onest[0:1, :]

            for g in range(2):
                for t in range(5):
                    # ---- A1: 8x8x32 pre-relu, M-tiles (jy, half) ----
                    h1 = apool.tile([128, 16, 405], f32, tag="h1")
                    for jy in range(8):
                        il = [i for i in (jy - 1, jy, jy + 1) if 0 <= i <= 7]
                        for h in range(2):
                            ps = ppool.tile([128, 405], f32, tag="ps")
                            for si, i in enumerate(il):
                                r0 = 18 * t + i
                                rhs = img[0:25, g, r0 // 2:r0 // 2 + 9, r0 % 2, :]
                                nc.tensor.matmul(
                                    ps[:], a1[:, jy, h, si, :], rhs,
                                    start=(si == 0), stop=(si == len(il) - 1),
                                )
                            nc.scalar.activation(
                                h1[:, jy * 2 + h, :], ps[:], AF.Relu,
                                bias=0.0, scale=1.0,
                            )
                    # ---- A2: 4x4x64, M-tiles (my, mx-pair) ----
                    h2 = apool.tile([128, 8, 405], f32, tag="h2")
                    for mt in range(8):
                        idxs = [n for n, e in enumerate(s2) if e[0] == mt]
                        ps = ppool.tile([128, 405], f32, tag="ps")
                        for k, n in enumerate(idxs):
                            nc.tensor.matmul(
                                ps[:], a2[:, n, :], h1[:, s2[n][1], :],
                                start=(k == 0), stop=False,
                            )
                        nc.tensor.matmul(
                            ps[:], bcol[0:1, 0, :], ones_rhs,
                            start=False, stop=True,
                        )
                        nc.scalar.activation(
                            h2[:, mt, :], ps[:], AF.Relu,
                            bias=0.0, scale=1.0,
                        )
                    # ---- A3: 2x2x128, M-tiles (ny,nx) ----
                    h3 = apool.tile([128, 4, 405], f32, tag="h3")
                    for mt in range(4):
                        idxs = [n for n, e in enumerate(s3) if e[0] == mt]
                        ps = ppool.tile([128, 405], f32, tag="ps")
                        for k, n in enumerate(idxs):
                            nc.tensor.matmul(
                                ps[:], a3[:, n, :], h2[:, s3[n][1], :],
                                start=(k == 0), stop=False,
                            )
                        nc.tensor.matmul(
                            ps[:], bcol[0:1, 1, :], ones_rhs,
                            start=False, stop=True,
                        )
                        nc.scalar.activation(
                            h3[:, mt, :], ps[:], AF.Relu,
                            bias=0.0, scale=1.0,
                        )
                    # ---- dense -> [v; v] then v_aug = [v; v^2] ----
                    psv = ppool.tile([128, 405], f32, tag="ps")
                    for pos in range(4):
                        nc.tensor.matmul(
                            psv[:], wdt[:, pos, :], h3[:, pos, :],
                            start=(pos == 0), stop=(pos == 3),
                        )
                    vaug = spool.tile([128, 405], f32, tag="vaug")
                    nc.scalar.activation(
                        vaug[0:64, :], psv[0:64, :], AF.Copy, bias=0.0, scale=1.0
                    )
                    nc.scalar.activation(
                        vaug[64:128, :], psv[64:128, :], AF.Square,
                        bias=0.0, scale=1.0,
                    )
                    # ---- RBF + exp + eps-floor + reduce ----
                    psr = ppool.tile([128, 405], f32, tag="ps")
                    nc.tensor.matmul(psr[:], rbf[:], vaug[:], start=True, stop=False)
                    nc.tensor.matmul(
                        psr[:], bcol[0:1, 2, :], ones_rhs,
                        start=False, stop=True,
                    )
                    ow = spool.tile([128, 405], f32, tag="ow")
                    nc.scalar.activation(
                        ow[:], psr[:], AF.Exp, bias=0.0, scale=1.0
                    )
                    # floor at EPS via relu(ow - EPS); ACT accum_out gives the
                    # per-partition sum over the 405 patches in one op
                    owr = spool.tile([128, 405], f32, tag="owr")
                    nc.scalar.activation(
                        owr[:], ow[:], AF.Relu, bias=-EPS, scale=1.0,
                        accum_out=red[:, g, t:t + 1],
                    )

            junk = apool.tile([128, 5], f32, tag="junk")
            for g in range(2):
                nc.scalar.activation(junk[:], red[:, g, :], AF.Copy,
                                     bias=0.0, scale=1.0,
                                     accum_out=S_raw.ap()[:, g:g + 1])
    # Final DMA outside the TileContext: the tile drain+barrier already
    # synced everything, so this needs no Tile-tracked waits (codegen here
    # allows only one sync-wait per instruction).
    with nc.semaphore("out_sem") as out_sem:
        nc.sync.dma_start(out_d[:], S_raw.ap()).then_inc(out_sem, 16)
        nc.sync.wait_ge(out_sem, 16)
    return nc


# ---------------------------------------------------------------- entry
def kernel(images, w1, b1, w2, b2, w3, b3, wd, bd, c_x, c_y, comp_w, sigma):
    from concourse.bass_utils import run_bass_kernel_spmd

    consts, _, _ = _build_consts(
        w1, b1, w2, b2, w3, b3, wd, bd, c_x, c_y, comp_w, sigma
    )
    key = "nc"
    if key not in _CACHE:
        _CACHE[key] = _build_nc(consts["a2c"].shape[1], consts["a3c"].shape[1])
    nc = _CACHE[key]

    images = np.asarray(images, F32)
    in_maps = _make_in_maps(images, consts)
    res = run_bass_kernel_spmd(nc, in_maps, core_ids=list(range(8)))
    return _postprocess(res, c_y)


def _make_in_maps(images, consts):
    n2 = consts["a2c"].shape[1]
    n3 = consts["a3c"].shape[1]
    wblob = np.concatenate([
        consts["a2c"].reshape(128, n2 * 128),
        consts["a3c"].reshape(128, n3 * 128),
        consts["wd2"].reshape(128, 512),
        consts["rbfw"],
        np.concatenate([consts["bcol"].reshape(1, 384),
                        np.zeros((127, 384), F32)], axis=0),
        np.concatenate([np.ones((1, 405), F32),
                        np.zeros((127, 405), F32)], axis=0),
    ], axis=1)
    wblob = np.ascontiguousarray(wblob, F32)
    in_maps = []
    for c in range(8):
        ext = np.empty((25, 2, 48, 2, 45), F32)
        ext[24] = 1.0
        for j in range(8):
            for ci in range(3):
                v = images[2 * c:2 * c + 2, :, j:j + 89:2, ci]   # (2,96,45)
                ext[j * 3 + ci] = v.reshape(2, 48, 2, 45)
        iblob = np.concatenate([
            consts["a1c"].reshape(25, 6144), ext.reshape(25, 8640)
        ], axis=1)
        m = {"wblob": wblob, "iblob": np.ascontiguousarray(iblob, F32)}
        in_maps.append(m)
    return in_maps


def _postprocess(res, c_y):
    S = np.zeros((16, 128), np.float64)
    for c in range(8):
        o = res.results[c]["out"]          # (128, 2)
        S[2 * c] = o[:, 0]
        S[2 * c + 1] = o[:, 1]
    S += NPI * EPS
    y_w = S / S.sum(-1, keepdims=True)
    cy = np.asarray(c_y, np.float64)
    y_v = cy / np.linalg.norm(cy, axis=-1, keepdims=True)
    probs = y_w @ (y_v ** 2)
    return np.ascontiguousarray(probs, F32)
